# revision 14
# baseline (speedup 1.0000x reference)
"""Causal self-attention (B=4, T=2048, C=1024, H=16, Dh=64) on 8 TRN2 NeuronCores.

Sharding: tensor-parallel over heads (2 heads per core) x all batches on every
core.  Matmul inputs are fp16 (fp32 PSUM accumulation); error vs the fp32
reference is ~2e-3.  Each core computes:
  - its 2 heads' Q^T/K^T via qkvT = W_qk^T @ x^T (PE) -> [Dh, seq] layout
  - V in [seq, Dh] layout via V = x @ W_v (lhsT = x^T tiles)
  - causal attention: ST[k,q] = K^T.T @ Q^T (2 heads row-tiled), exp on ACT
    (both heads in one [128,1024] op), denominator via a ones-column appended
    to V (AV matmul M=65), reciprocal + gpsimd partition-broadcast for the
    softmax division
  - partial output projection out_p = Y_local @ W_p_rows (row-parallel)
Host side: x transpose + fp16 cast, weight slicing, partial-sum + bias.
"""

import sys

if "/opt/trn_rl_repo" not in sys.path:
    sys.path.insert(0, "/opt/trn_rl_repo")

import numpy as np

B, T, C, H, Dh = 4, 2048, 1024, 16, 64
NCORES = 8
HPC = H // NCORES          # heads per core = 2
M = B * T                  # 8192 rows
KT_C = C // 128            # 8 contraction tiles for the projections
TKT = T // 128             # 16 key tiles per batch
QC = T // 512              # 4 query chunks of 512 per batch
SCALE = 1.0 / np.sqrt(Dh)

_cache = {}


def _build(has_v_bias: bool):
    import concourse.tile as tile
    from concourse import bacc, mybir

    f32 = mybir.dt.float32
    f16 = mybir.dt.float16
    EXP = mybir.ActivationFunctionType.Exp

    nc = bacc.Bacc("TRN2", target_bir_lowering=False, debug=False,
                   num_devices=NCORES)

    xT_d = nc.dram_tensor("xT", [C, M], f16, kind="ExternalInput")
    wqk_d = nc.dram_tensor("w_qk", [C, 2 * HPC * Dh], f16, kind="ExternalInput")
    wv_d = nc.dram_tensor("w_v", [C, HPC * Dh], f16, kind="ExternalInput")
    wp_d = nc.dram_tensor("w_p", [HPC * Dh, C], f16, kind="ExternalInput")
    bqk_d = nc.dram_tensor("b_qk", [128, 2], f32, kind="ExternalInput")
    bv_d = nc.dram_tensor("b_v_row", [128, HPC * Dh], f32, kind="ExternalInput")
    mask_d = nc.dram_tensor("masks", [128, 4, 2, 512], f16, kind="ExternalInput")
    out_d = nc.dram_tensor("out_p", [M, C], f32, kind="ExternalOutput")

    xT_t = xT_d.ap().rearrange("(kt p) m -> p kt m", p=128)   # [128, 8, 8192]
    wqk_t = wqk_d.ap().rearrange("(kt p) n -> p kt n", p=128)  # [128, 8, 256]
    wv_t = wv_d.ap().rearrange("(kt p) n -> p kt n", p=128)    # [128, 8, 128]

    with tile.TileContext(nc) as tc:
        with tc.tile_pool(name="consts", bufs=1) as consts, \
             tc.tile_pool(name="work", bufs=2) as work, \
             tc.tile_pool(name="pbuf", bufs=12) as pbuf, \
             tc.tile_pool(name="obuf", bufs=4) as obuf, \
             tc.tile_pool(name="psum", bufs=2, space="PSUM") as psum, \
             tc.tile_pool(name="psst", bufs=2, space="PSUM") as psst, \
             tc.tile_pool(name="psyt", bufs=2, space="PSUM") as psyt:

            # ---- constants ----
            wqk_sb = consts.tile([128, KT_C, 2 * HPC * Dh], f16)
            nc.sync.dma_start(wqk_sb[:], wqk_t)
            wv_sb = consts.tile([128, KT_C, HPC * Dh], f16)
            nc.sync.dma_start(wv_sb[:], wv_t)
            wp_sb = consts.tile([128, C], f16)
            nc.sync.dma_start(wp_sb[:], wp_d.ap())
            bqk_sb = consts.tile([128, 2], f32)
            nc.sync.dma_start(bqk_sb[:], bqk_d.ap())
            mask_sb = consts.tile([128, 4, 2, 512], f16)
            nc.sync.dma_start(mask_sb[:], mask_d.ap())
            if has_v_bias:
                bv_sb = consts.tile([128, HPC * Dh], f32)
                nc.sync.dma_start(bv_sb[:], bv_d.ap())

            for b in range(B):
                m0 = b * T

                # ---------- QKV projection for batch b ----------
                QT = work.tile([128, T], f16, tag="QT")   # rows 0-63 h0, 64-127 h1
                KTt = work.tile([128, T], f16, tag="KT")
                Vt = work.tile([128, TKT, 2 * Dh + 2], f16, tag="Vt")
                # ones columns for the softmax denominator (cols 64 and 129)
                nc.vector.memset(Vt[:, :, Dh:Dh + 1], 1.0)
                nc.vector.memset(Vt[:, :, 2 * Dh + 1:2 * Dh + 2], 1.0)

                for mc in range(4):                      # 512-row chunks
                    ms0 = m0 + mc * 512
                    xt = work.tile([128, KT_C, 512], f16, tag="xt", bufs=5)
                    nc.sync.dma_start(xt[:], xT_t[:, :, ms0:ms0 + 512])

                    for nt, dest in ((0, QT), (1, KTt)):
                        ps = psum.tile([128, 512], f32, tag="ps")
                        for kt in range(KT_C):
                            nc.tensor.matmul(
                                ps[:],
                                wqk_sb[:, kt, nt * 128:(nt + 1) * 128],
                                xt[:, kt, :],
                                start=(kt == 0), stop=(kt == KT_C - 1))
                        nc.vector.tensor_scalar_add(
                            dest[:, mc * 512:(mc + 1) * 512], ps[:],
                            bqk_sb[:, nt:nt + 1])

                    for msl in range(4):                 # V for 128-row slices
                        vps = psum.tile([128, 128], f32, tag="ps")
                        for kt in range(KT_C):
                            nc.tensor.matmul(
                                vps[:],
                                xt[:, kt, msl * 128:(msl + 1) * 128],
                                wv_sb[:, kt, :],
                                start=(kt == 0), stop=(kt == KT_C - 1))
                        ktile = mc * 4 + msl
                        if has_v_bias:
                            nc.vector.tensor_add(
                                Vt[:, ktile, 0:Dh], vps[:, 0:Dh],
                                bv_sb[:, 0:Dh])
                            nc.vector.tensor_add(
                                Vt[:, ktile, Dh + 1:2 * Dh + 1],
                                vps[:, Dh:2 * Dh], bv_sb[:, Dh:2 * Dh])
                        else:
                            nc.vector.tensor_copy(
                                Vt[:, ktile, 0:Dh], vps[:, 0:Dh])
                            nc.vector.tensor_copy(
                                Vt[:, ktile, Dh + 1:2 * Dh + 1],
                                vps[:, Dh:2 * Dh])

                # ---------- causal attention for batch b ----------
                YT = work.tile([128, T], f16, tag="YT")
                for qc in range(QC):
                    q_sl = slice(qc * 512, (qc + 1) * 512)
                    nkt = 4 * (qc + 1)
                    yt0 = psyt.tile([65, 512], f32, tag="yt")
                    yt1 = psyt.tile([65, 512], f32, tag="yt")
                    for kt in range(nkt):
                        k_sl = slice(kt * 128, (kt + 1) * 128)
                        stp = psst.tile([128, 2, 512], f32, tag="st")
                        # ST[k, q] = K^T(h)[d, k].T @ Q^T(h)[d, q]; the two
                        # heads run concurrently on PE row-groups 0-63/64-127
                        nc.tensor.matmul(stp[:, 0, :], KTt[0:64, k_sl],
                                         QT[0:64, q_sl], start=True, stop=True)
                        nc.tensor.matmul(stp[:, 1, :], KTt[64:128, k_sl],
                                         QT[64:128, q_sl], start=True, stop=True)
                        pp = pbuf.tile([128, 2, 512], f16, tag="pp")
                        nc.scalar.activation(pp[:], stp[:], EXP, scale=SCALE)
                        r = kt - 4 * qc
                        if r >= 0:                        # diagonal: mask
                            nc.vector.tensor_mul(pp[:], pp[:], mask_sb[:, r])
                        first, last = (kt == 0), (kt == nkt - 1)
                        nc.tensor.matmul(yt0[:], Vt[:, kt, 0:Dh + 1],
                                         pp[:, 0, :], start=first, stop=last)
                        nc.tensor.matmul(yt1[:], Vt[:, kt, Dh + 1:2 * Dh + 2],
                                         pp[:, 1, :], start=first, stop=last)
                    # softmax division: Y[d, q] * (1 / Z[q])
                    for h, ytp in ((0, yt0), (1, yt1)):
                        rec = obuf.tile([1, 512], f32, tag="rec")
                        nc.vector.reciprocal(rec[:], ytp[64:65, :])
                        bc = obuf.tile([64, 512], f32, tag="bc")
                        nc.gpsimd.partition_broadcast(bc[:], rec[:])
                        nc.vector.tensor_mul(
                            YT[h * 64:(h + 1) * 64, q_sl],
                            ytp[0:64, :], bc[:])

                    # ---- project this q-chunk (row-parallel partial) ----
                    for mt in range(4 * qc, 4 * qc + 4):
                        for nh in range(2):
                            pp2 = psum.tile([128, 512], f32, tag="ps")
                            nc.tensor.matmul(
                                pp2[:], YT[:, mt * 128:(mt + 1) * 128],
                                wp_sb[:, nh * 512:(nh + 1) * 512],
                                start=True, stop=True)
                            ot = obuf.tile([128, 512], f32, tag="ot", bufs=6)
                            nc.vector.tensor_copy(ot[:], pp2[:])
                            nc.sync.dma_start(
                                out_d.ap()[m0 + mt * 128:m0 + (mt + 1) * 128,
                                           nh * 512:(nh + 1) * 512],
                                ot[:])

    nc.compile()
    return nc


def _get_nc(has_v_bias: bool):
    key = ("nc", has_v_bias)
    if key not in _cache:
        _cache[key] = _build(has_v_bias)
    return _cache[key]


def _make_masks() -> np.ndarray:
    # masks[p, r, h, q] = 1.0 where key (128*r + p) <= query q in a 512-chunk
    p = np.arange(128)[:, None, None]
    r = np.arange(4)[None, :, None]
    q = np.arange(512)[None, None, :]
    m = ((128 * r + p) <= q).astype(np.float16)           # [128, 4, 512]
    return np.ascontiguousarray(np.repeat(m[:, :, None, :], 2, axis=2))


def kernel(x, W_qkv, b_qkv, W_proj, b_proj):
    from concourse.bass_utils import run_bass_kernel_spmd

    x = np.asarray(x, dtype=np.float32)
    W_qkv = np.asarray(W_qkv, dtype=np.float32)
    b_qkv = np.asarray(b_qkv, dtype=np.float32)
    W_proj = np.asarray(W_proj, dtype=np.float32)
    b_proj = np.asarray(b_proj, dtype=np.float32)

    has_v_bias = bool(np.any(b_qkv[2 * C:] != 0.0))
    nc = _get_nc(has_v_bias)

    xT = np.ascontiguousarray(x.reshape(M, C).T.astype(np.float16))
    masks = _make_masks()

    in_maps = []
    for c in range(NCORES):
        h0 = HPC * c * Dh                                  # channel offset
        w_q = W_qkv[:, h0:h0 + HPC * Dh]
        w_k = W_qkv[:, C + h0:C + h0 + HPC * Dh]
        w_v = W_qkv[:, 2 * C + h0:2 * C + h0 + HPC * Dh]
        b_q = b_qkv[h0:h0 + HPC * Dh]
        b_k = b_qkv[C + h0:C + h0 + HPC * Dh]
        b_v = b_qkv[2 * C + h0:2 * C + h0 + HPC * Dh]
        in_maps.append({
            "xT": xT,
            "w_qk": np.ascontiguousarray(
                np.concatenate([w_q, w_k], axis=1).astype(np.float16)),
            "w_v": np.ascontiguousarray(w_v.astype(np.float16)),
            "w_p": np.ascontiguousarray(
                W_proj[h0:h0 + HPC * Dh, :].astype(np.float16)),
            "b_qk": np.ascontiguousarray(np.stack([b_q, b_k], axis=1)),
            "b_v_row": np.ascontiguousarray(
                np.broadcast_to(b_v[None, :], (128, HPC * Dh))),
            "masks": masks,
        })

    res = run_bass_kernel_spmd(nc, in_maps, core_ids=list(range(NCORES)),
                               **_cache.get("run_kwargs", {}))
    _cache["last_results"] = res

    acc = np.zeros((M, C), dtype=np.float64)
    for c in range(NCORES):
        acc += res.results[c]["out_p"]
    acc += b_proj
    return acc.astype(np.float32).reshape(B, T, C)


# revision 15
# speedup vs baseline: 1.0703x; 1.0703x over previous
"""Causal self-attention (B=4, T=2048, C=1024, H=16, Dh=64) on 8 TRN2 NeuronCores.

Sharding: tensor-parallel over heads (2 heads per core) x all batches on every
core.  Matmul inputs are fp16 (fp32 PSUM accumulation); error vs the fp32
reference is ~2e-3.  Each core computes:
  - its 2 heads' Q^T/K^T via qkvT = W_qk^T @ x^T (PE) -> [Dh, seq] layout
  - V in [seq, Dh] layout via V = x @ W_v (lhsT = x^T tiles)
  - causal attention: ST[k,q] = K^T.T @ Q^T (2 heads row-tiled), exp on ACT
    (both heads in one [128,1024] op), denominator via a ones-column appended
    to V (AV matmul M=65), reciprocal + gpsimd partition-broadcast for the
    softmax division
  - partial output projection out_p = Y_local @ W_p_rows (row-parallel)
Host side: x transpose + fp16 cast, weight slicing, partial-sum + bias.
"""

import sys

if "/opt/trn_rl_repo" not in sys.path:
    sys.path.insert(0, "/opt/trn_rl_repo")

import numpy as np

B, T, C, H, Dh = 4, 2048, 1024, 16, 64
NCORES = 8
HPC = H // NCORES          # heads per core = 2
M = B * T                  # 8192 rows
KT_C = C // 128            # 8 contraction tiles for the projections
TKT = T // 128             # 16 key tiles per batch
QC = T // 512              # 4 query chunks of 512 per batch
SCALE = 1.0 / np.sqrt(Dh)

_cache = {}


def _build(has_v_bias: bool):
    import concourse.tile as tile
    from concourse import bacc, mybir

    f32 = mybir.dt.float32
    f16 = mybir.dt.float16
    EXP = mybir.ActivationFunctionType.Exp

    nc = bacc.Bacc("TRN2", target_bir_lowering=False, debug=False,
                   num_devices=NCORES)

    xT_d = nc.dram_tensor("xT", [C, M], f16, kind="ExternalInput")
    wqk_d = nc.dram_tensor("w_qk", [C, 2 * HPC * Dh], f16, kind="ExternalInput")
    wv_d = nc.dram_tensor("w_v", [C, HPC * Dh], f16, kind="ExternalInput")
    wp_d = nc.dram_tensor("w_p", [HPC * Dh, C], f16, kind="ExternalInput")
    bqk_d = nc.dram_tensor("b_qk", [128, 2], f32, kind="ExternalInput")
    bv_d = nc.dram_tensor("b_v_row", [128, HPC * Dh], f32, kind="ExternalInput")
    mask_d = nc.dram_tensor("masks", [128, 4, 2, 512], f16, kind="ExternalInput")
    out_d = nc.dram_tensor("out_p", [M, C], f32, kind="ExternalOutput")

    xT_t = xT_d.ap().rearrange("(kt p) m -> p kt m", p=128)   # [128, 8, 8192]
    wqk_t = wqk_d.ap().rearrange("(kt p) n -> p kt n", p=128)  # [128, 8, 256]
    wv_t = wv_d.ap().rearrange("(kt p) n -> p kt n", p=128)    # [128, 8, 128]

    with tile.TileContext(nc) as tc:
        with tc.tile_pool(name="consts", bufs=1) as consts, \
             tc.tile_pool(name="work", bufs=2) as work, \
             tc.tile_pool(name="pbuf", bufs=12) as pbuf, \
             tc.tile_pool(name="obuf", bufs=4) as obuf, \
             tc.tile_pool(name="psum", bufs=2, space="PSUM") as psum, \
             tc.tile_pool(name="psst", bufs=2, space="PSUM") as psst, \
             tc.tile_pool(name="psyt", bufs=2, space="PSUM") as psyt:

            # ---- constants ----
            wqk_sb = consts.tile([128, KT_C, 2 * HPC * Dh], f16)
            nc.sync.dma_start(wqk_sb[:], wqk_t)
            wv_sb = consts.tile([128, KT_C, HPC * Dh], f16)
            nc.sync.dma_start(wv_sb[:], wv_t)
            wp_sb = consts.tile([128, C], f16)
            nc.sync.dma_start(wp_sb[:], wp_d.ap())
            bqk_sb = consts.tile([128, 2], f32)
            nc.sync.dma_start(bqk_sb[:], bqk_d.ap())
            mask_sb = consts.tile([128, 4, 2, 512], f16)
            nc.sync.dma_start(mask_sb[:], mask_d.ap())
            if has_v_bias:
                bv_sb = consts.tile([128, HPC * Dh], f32)
                nc.sync.dma_start(bv_sb[:], bv_d.ap())

            for b in range(B):
                m0 = b * T

                # ---------- QKV projection for batch b ----------
                QT = work.tile([128, T], f16, tag="QT")   # rows 0-63 h0, 64-127 h1
                KTt = work.tile([128, T], f16, tag="KT")
                Vt = work.tile([128, TKT, 2 * Dh + 2], f16, tag="Vt")
                # ones columns for the softmax denominator (cols 64 and 129)
                nc.vector.memset(Vt[:, :, Dh:Dh + 1], 1.0)
                nc.vector.memset(Vt[:, :, 2 * Dh + 1:2 * Dh + 2], 1.0)

                for mc in range(4):                      # 512-row chunks
                    ms0 = m0 + mc * 512
                    xt = work.tile([128, KT_C, 512], f16, tag="xt", bufs=5)
                    nc.sync.dma_start(xt[:], xT_t[:, :, ms0:ms0 + 512])

                    for nt, dest in ((0, QT), (1, KTt)):
                        ps = psum.tile([128, 512], f32, tag="ps")
                        for kt in range(KT_C):
                            nc.tensor.matmul(
                                ps[:],
                                wqk_sb[:, kt, nt * 128:(nt + 1) * 128],
                                xt[:, kt, :],
                                start=(kt == 0), stop=(kt == KT_C - 1))
                        nc.vector.tensor_scalar_add(
                            dest[:, mc * 512:(mc + 1) * 512], ps[:],
                            bqk_sb[:, nt:nt + 1])

                    for msl in range(4):                 # V for 128-row slices
                        vps = psum.tile([128, 128], f32, tag="ps")
                        for kt in range(KT_C):
                            nc.tensor.matmul(
                                vps[:],
                                xt[:, kt, msl * 128:(msl + 1) * 128],
                                wv_sb[:, kt, :],
                                start=(kt == 0), stop=(kt == KT_C - 1))
                        ktile = mc * 4 + msl
                        if has_v_bias:
                            nc.vector.tensor_add(
                                Vt[:, ktile, 0:Dh], vps[:, 0:Dh],
                                bv_sb[:, 0:Dh])
                            nc.vector.tensor_add(
                                Vt[:, ktile, Dh + 1:2 * Dh + 1],
                                vps[:, Dh:2 * Dh], bv_sb[:, Dh:2 * Dh])
                        else:
                            nc.vector.tensor_copy(
                                Vt[:, ktile, 0:Dh], vps[:, 0:Dh])
                            nc.vector.tensor_copy(
                                Vt[:, ktile, Dh + 1:2 * Dh + 1],
                                vps[:, Dh:2 * Dh])

                # ---------- causal attention for batch b ----------
                YT = work.tile([128, T], f16, tag="YT")
                for qc in range(QC):
                    q_sl = slice(qc * 512, (qc + 1) * 512)
                    nkt = 4 * (qc + 1)
                    yt0 = psyt.tile([65, 512], f32, tag="yt")
                    yt1 = psyt.tile([65, 512], f32, tag="yt")
                    for kt in range(nkt):
                        k_sl = slice(kt * 128, (kt + 1) * 128)
                        stp = psst.tile([128, 2, 512], f32, tag="st")
                        # ST[k, q] = K^T(h)[d, k].T @ Q^T(h)[d, q]; the two
                        # heads run concurrently on PE row-groups 0-63/64-127
                        nc.tensor.matmul(stp[:, 0, :], KTt[0:64, k_sl],
                                         QT[0:64, q_sl], start=True, stop=True)
                        nc.tensor.matmul(stp[:, 1, :], KTt[64:128, k_sl],
                                         QT[64:128, q_sl], start=True, stop=True)
                        pp = pbuf.tile([128, 2, 512], f16, tag="pp")
                        nc.scalar.activation(pp[:], stp[:], EXP, scale=SCALE)
                        r = kt - 4 * qc
                        if r >= 0:                        # diagonal: mask
                            nc.vector.tensor_mul(pp[:], pp[:], mask_sb[:, r])
                        first, last = (kt == 0), (kt == nkt - 1)
                        nc.tensor.matmul(yt0[:], Vt[:, kt, 0:Dh + 1],
                                         pp[:, 0, :], start=first, stop=last)
                        nc.tensor.matmul(yt1[:], Vt[:, kt, Dh + 1:2 * Dh + 2],
                                         pp[:, 1, :], start=first, stop=last)
                    # softmax division: Y[d, q] * (1 / Z[q])
                    for h, ytp in ((0, yt0), (1, yt1)):
                        rec = obuf.tile([1, 512], f32, tag="rec")
                        nc.vector.reciprocal(rec[:], ytp[64:65, :])
                        bc = obuf.tile([64, 512], f32, tag="bc")
                        nc.gpsimd.partition_broadcast(bc[:], rec[:])
                        nc.vector.tensor_mul(
                            YT[h * 64:(h + 1) * 64, q_sl],
                            ytp[0:64, :], bc[:])

                # ---------- output projection (row-parallel partial) ----------
                for mt in range(TKT):
                    for nh in range(2):
                        pp2 = psum.tile([128, 512], f32, tag="ps")
                        nc.tensor.matmul(
                            pp2[:], YT[:, mt * 128:(mt + 1) * 128],
                            wp_sb[:, nh * 512:(nh + 1) * 512],
                            start=True, stop=True)
                        ot = obuf.tile([128, 512], f32, tag="ot", bufs=6)
                        nc.vector.tensor_copy(ot[:], pp2[:])
                        nc.sync.dma_start(
                            out_d.ap()[m0 + mt * 128:m0 + (mt + 1) * 128,
                                       nh * 512:(nh + 1) * 512],
                            ot[:])



    nc.compile()
    return nc


def _get_nc(has_v_bias: bool):
    key = ("nc", has_v_bias)
    if key not in _cache:
        _cache[key] = _build(has_v_bias)
    return _cache[key]


def _make_masks() -> np.ndarray:
    # masks[p, r, h, q] = 1.0 where key (128*r + p) <= query q in a 512-chunk
    p = np.arange(128)[:, None, None]
    r = np.arange(4)[None, :, None]
    q = np.arange(512)[None, None, :]
    m = ((128 * r + p) <= q).astype(np.float16)           # [128, 4, 512]
    return np.ascontiguousarray(np.repeat(m[:, :, None, :], 2, axis=2))


def kernel(x, W_qkv, b_qkv, W_proj, b_proj):
    from concourse.bass_utils import run_bass_kernel_spmd

    x = np.asarray(x, dtype=np.float32)
    W_qkv = np.asarray(W_qkv, dtype=np.float32)
    b_qkv = np.asarray(b_qkv, dtype=np.float32)
    W_proj = np.asarray(W_proj, dtype=np.float32)
    b_proj = np.asarray(b_proj, dtype=np.float32)

    has_v_bias = bool(np.any(b_qkv[2 * C:] != 0.0))
    nc = _get_nc(has_v_bias)

    xT = np.ascontiguousarray(x.reshape(M, C).T.astype(np.float16))
    masks = _make_masks()

    in_maps = []
    for c in range(NCORES):
        h0 = HPC * c * Dh                                  # channel offset
        w_q = W_qkv[:, h0:h0 + HPC * Dh]
        w_k = W_qkv[:, C + h0:C + h0 + HPC * Dh]
        w_v = W_qkv[:, 2 * C + h0:2 * C + h0 + HPC * Dh]
        b_q = b_qkv[h0:h0 + HPC * Dh]
        b_k = b_qkv[C + h0:C + h0 + HPC * Dh]
        b_v = b_qkv[2 * C + h0:2 * C + h0 + HPC * Dh]
        in_maps.append({
            "xT": xT,
            "w_qk": np.ascontiguousarray(
                np.concatenate([w_q, w_k], axis=1).astype(np.float16)),
            "w_v": np.ascontiguousarray(w_v.astype(np.float16)),
            "w_p": np.ascontiguousarray(
                W_proj[h0:h0 + HPC * Dh, :].astype(np.float16)),
            "b_qk": np.ascontiguousarray(np.stack([b_q, b_k], axis=1)),
            "b_v_row": np.ascontiguousarray(
                np.broadcast_to(b_v[None, :], (128, HPC * Dh))),
            "masks": masks,
        })

    res = run_bass_kernel_spmd(nc, in_maps, core_ids=list(range(NCORES)),
                               **_cache.get("run_kwargs", {}))
    _cache["last_results"] = res

    acc = np.zeros((M, C), dtype=np.float64)
    for c in range(NCORES):
        acc += res.results[c]["out_p"]
    acc += b_proj
    return acc.astype(np.float32).reshape(B, T, C)
